# revision 21
# baseline (speedup 1.0000x reference)
"""Trainium2 Bass kernel for CrossAttention — v4: projection folding.

reference math:
    out = softmax((hs Wq)(dec Wkv_lo)^T / sqrt(D)) @ (dec Wkv_hi)

Associativity lets both K and V vanish:
    W_qk = Wq @ Wkv_lo^T          (host, fp32, [D, D])
    A    = hs @ W_qk              # phase B'  [QS, D]   1.07 GMAC
    S    = A @ dec^T / sqrt(D)    # phase C1  [QS, KL]  2.15 GMAC (as S^T on PE)
    P    = exp(S)
    U    = P @ dec                # phase C2  [QS, D]   2.15 GMAC (as U^T on PE)
    out  = (U @ Wkv_hi) / rowsum  # phase C3  [QS, D]   1.07 GMAC

Per-core 6.44 GMAC (was 9.66 duplicated-KV) = 164us ideal PE @ 2.4GHz bf16.
No collectives: dec is an input, so every core just loads the full dec in
both layouts (d-major for C1's stationary, k-major for C2's stationary).

Sharding: 8 cores = batch(4) x q-half(2), embarrassingly parallel.
All matmuls bf16 (host-rounded), fp32 PSUM.  scores^T on the PE (512-wide q
moving dim); exp() output is directly the C2 stationary; row sums via DVE
kt-tree + one 1-wide ones-matmul per 128-q chunk; 1/rowsum applied at C3's
PSUM->SBUF output copy.

PSUM: one shared 4-buffer ring serves B'/C1/C2/C3 (C2 runs dj-outer so each
U^T chain drains before the next starts) — no pool close/open barriers at
phase transitions.  Emission order C1(0) C1(1) sums(0) C2(0) sums(1) C2(1)
C3(0) C3(1) keeps every PE instruction's deps ~27us ahead.

This walrus build allows only ONE embedded semaphore wait per hardware
instruction; legalize_waits() splits extra waits onto same-engine NOPs.
"""

import sys

if "/opt/trn_rl_repo" not in sys.path:
    sys.path.insert(0, "/opt/trn_rl_repo")

import numpy as np
import ml_dtypes

import bass_rust
import concourse.bass as bass
import concourse.mybir as mybir
import concourse.tile as tile
from concourse.bass_utils import run_bass_kernel_spmd

F32 = mybir.dt.float32
BF16 = mybir.dt.bfloat16
EXP = mybir.ActivationFunctionType.Exp
ACOPY = mybir.ActivationFunctionType.Copy

N_CORES = 8
B, QL, KL, D = 4, 2048, 2048, 1024
NWARM = 8


def legalize_waits(nc, max_waits=1):
    cnt = 0
    for fn in nc.m.functions:
        for bb in fn.blocks:
            out = []
            changed = False
            for ins in bb.instructions:
                si = ins.sync_info
                if si is not None and si.on_wait and len(si.on_wait) > max_waits:
                    waits = list(si.on_wait)
                    for w in waits[:-max_waits]:
                        cnt += 1
                        nop = bass_rust.InstNoOp(name=f"I-wfix-{cnt}")
                        nop.engine = ins.engine
                        nop.sync_info = mybir.SyncInfo(on_wait=[w], on_update=[])
                        out.append(nop)
                    ins.sync_info = mybir.SyncInfo(
                        on_wait=waits[-max_waits:],
                        on_update=list(si.on_update or []),
                    )
                    changed = True
                out.append(ins)
            if changed:
                bb.instructions = out
    return cnt


def build_attention(nc, QS, KLp, Dp, scale):
    DS = Dp // 128          # d 128-chunks / contraction subtiles (8)
    NKT = KLp // 128        # k 128-tiles (16)
    NKG = NKT // 4          # decT 4-block groups (4)
    NQB = QS // 512         # q 512-blocks (2)
    NDC = Dp // 512         # d 512-chunks (2)
    BLK = DS * 128

    hsT = nc.declare_dram_parameter("hsT", [QS // 128, 128, BLK], BF16, isOutput=False)
    decT = nc.declare_dram_parameter("decT", [NKT, 128, BLK], BF16, isOutput=False)
    deck = nc.declare_dram_parameter("deck", [NKT, 128, Dp], BF16, isOutput=False)
    wqk = nc.declare_dram_parameter("wqk", [DS, 128, BLK], BF16, isOutput=False)
    whiP = nc.declare_dram_parameter("whi", [DS, 128, BLK], BF16, isOutput=False)
    # bf16 output: host upcasts; halves the output DMA and the final-chunk tail
    out = nc.declare_dram_parameter("out", [QS, Dp], BF16, isOutput=True)

    def load_blocks(dst, src, blk0, nblk):
        if nblk == 1:
            nc.sync.dma_start(
                dst[:], src[blk0].rearrange("p (s o) -> p s o", o=128)
            )
        else:
            nc.sync.dma_start(
                dst.rearrange("p b s o -> p b (s o)"),
                src[blk0 : blk0 + nblk].rearrange("b p f -> p b f"),
            )

    with tile.TileContext(nc) as tc:
        pools = []

        def enter(cm):
            pools.append(cm)
            return cm.__enter__()

        def close(cm):
            pools.remove(cm)
            cm.__exit__(None, None, None)

        # ---- long-lived pools (right stack) ----
        constp_cm = tc.tile_pool(name="const", bufs=1, side="right")
        atp_cm = tc.tile_pool(name="atp", bufs=1, side="right")
        dtp_cm = tc.tile_pool(name="dtp", bufs=4, side="right")
        dkp_cm = tc.tile_pool(name="dkp", bufs=1, side="right")
        whip_cm = tc.tile_pool(name="whi", bufs=1, side="right")
        constp = enter(constp_cm)
        atp = enter(atp_cm)
        dtp = enter(dtp_cm)
        dkp = enter(dkp_cm)
        whip = enter(whip_cm)

        AT = atp.tile([128, DS, QS], BF16, tag="AT")          # [d, di, q]
        dts = [
            dtp.tile([128, 4, DS, 128], BF16, tag="dtp", name=f"dt{g}")
            for g in range(NKG)
        ]                                                     # dec, d-major
        DK = dkp.tile([128, NKT, Dp], BF16, tag="DK")         # dec, k-major
        whi = whip.tile([128, DS, DS, 128], BF16, tag="whi")  # Wkv_hi blocks
        ones = constp.tile([128, 1], BF16)
        nc.gpsimd.memset(ones[:], 1.0)

        # ---- transient pools (left stack, opened in reverse close order) ----
        wqp_cm = tc.tile_pool(name="wqp", bufs=1)
        htp_cm = tc.tile_pool(name="hst", bufs=4)
        # ONE psum ring for every phase (B', C1, C2, C3): with C2 dj-outer no
        # phase needs >3 banks live, and sharing the pool removes the
        # close/open drain barrier (~0.8us) at each phase transition
        psM_cm = tc.tile_pool(name="psM", bufs=4, space="PSUM")
        psSum_cm = tc.tile_pool(name="psSum", bufs=2, space="PSUM")
        wqp = enter(wqp_cm)
        htp = enter(htp_cm)
        psB = enter(psM_cm)
        psS = psU = psO = psB
        psSum = enter(psSum_cm)

        # Clock-ramp warmup: the PE pstate resets to mid speed after ANY
        # >100ns idle, so the warmup must run CONTINUOUSLY until the first
        # chain's data lands (~12us) and end as close to that as possible
        # (the in-order PE queue means overshoot delays real work 1:1).
        # 8 x 512-wide cover the ramp; a tail of 128-wide matmuls (~50-100ns
        # each) gives fine-grained landing so the idle stays under 100ns.
        warm = constp.tile([128, 640], BF16)
        nc.gpsimd.memset(warm[:], 1.0)
        warm_ps_cm = tc.tile_pool(name="wps", bufs=1, space="PSUM")
        warm_ps_pool = enter(warm_ps_cm)
        warm_ps = warm_ps_pool.tile([128, 512], F32)
        for _ in range(8):
            nc.tensor.matmul(
                warm_ps[:], warm[:, 0:128], warm[:, 128:640],
                start=True, stop=True, skip_group_check=True,
            )
        for _ in range(6):
            nc.tensor.matmul(
                warm_ps[:, 0:128], warm[:, 0:128], warm[:, 128:256],
                start=True, stop=True, skip_group_check=True,
            )
        close(warm_ps_cm)

        # ---- critical-first loads: B's first half-chain, then background ---
        # hs tiles split in 2-block halves so the very first chain needs only
        # wqk block0 (0.25MB) + 2 hs blocks (0.5MB) of DMA
        wqt = wqp.tile([128, DS, DS, 128], BF16, tag="wqp")
        load_blocks(wqt[:, 0:1], wqk, 0, 1)
        hts = []
        for i in range(2 * NQB):
            hts.append(htp.tile([128, 2, DS, 128], BF16, tag="hst", name=f"ht{i}"))
        load_blocks(hts[0][:], hsT, 0, 2)
        load_blocks(hts[1][:], hsT, 2, 2)
        load_blocks(wqt[:, 1:2], wqk, 1, 1)
        load_blocks(wqt[:, 2:5], wqk, 2, 3)
        load_blocks(hts[2][:], hsT, 4, 2)
        load_blocks(hts[3][:], hsT, 6, 2)
        load_blocks(wqt[:, 5:DS], wqk, 5, DS - 5)

        # ------------- Phase B': AT[d, q] = W_qk^T @ hs^T -------------------
        # 256-wide half-chains (bf16 streams 1 row/cycle regardless of width)
        for qc in range(NQB):
            for do in range(DS):
                if qc == 0:
                    # C1's dec blocks, behind B's critical loads
                    if do == 2:
                        load_blocks(dts[0][:], decT, 0, 4)
                    elif do == 4:
                        load_blocks(dts[1][:], decT, 4, 4)
                    elif do == 6:
                        load_blocks(dts[2][:], decT, 8, 4)
                else:
                    if do == 0:
                        load_blocks(dts[3][:], decT, 12, 4)
                    elif do == 2:
                        # C2's k-major dec: one big DMA, 4MB
                        nc.sync.dma_start(
                            DK.rearrange("p t f -> p t f"),
                            deck.rearrange("t p f -> p t f"),
                        )
                    elif do == 6:
                        load_blocks(whi[:], whiP, 0, DS)
                ps = psB.tile([128, 512], F32, tag="psM")
                for h in range(2):
                    ht = hts[2 * qc + h]
                    for di in range(DS):
                        nc.tensor.matmul(
                            ps[:, h * 256 : (h + 1) * 256],
                            wqt[:, do, di, :], ht[:, :, di, :],
                            start=(di == 0), stop=(di == DS - 1),
                            skip_group_check=True,
                        )
                nc.vector.tensor_copy(AT[:, do, qc * 512 : (qc + 1) * 512], ps[:])
        close(htp_cm)
        close(wqp_cm)

        # ------------- Phase C: attention ------------------------------------
        ptp_cm = tc.tile_pool(name="ptp", bufs=2, side="right")
        trp_cm = tc.tile_pool(name="trp", bufs=2, side="right")
        statp_cm = tc.tile_pool(name="stat", bufs=2, side="right")
        utp_cm = tc.tile_pool(name="utp", bufs=2, side="right")
        ostp_cm = tc.tile_pool(name="ost", bufs=3, side="right")
        ptp = enter(ptp_cm)
        trp = enter(trp_cm)
        statp = enter(statp_cm)
        utp = enter(utp_cm)
        ostp = enter(ostp_cm)

        PTs, PTsums, recss = [], [], []

        def emit_scores(qb):
            """C1: PT[k, kt, q] = exp(scale * dec @ A^T) for one 512-q block,
            plus the DVE row-sum tree."""
            PT = ptp.tile([128, NKT, 512], BF16, tag="ptp", name=f"PT{qb}")
            for kt in range(NKT):
                ps = psS.tile([128, 512], F32, tag="psM")
                for di in range(DS):
                    nc.tensor.matmul(
                        ps[:], dts[kt // 4][:, kt % 4, di, :],
                        AT[:, di, qb * 512 : (qb + 1) * 512],
                        start=(di == 0), stop=(di == DS - 1),
                    )
                nc.scalar.activation(
                    PT[:, kt, :], ps[:], EXP, bias=0.0, scale=float(scale)
                )
            t8 = trp.tile([128, 8, 512], BF16, tag="t8", name=f"t8_{qb}")
            nc.vector.tensor_tensor(
                t8[:], PT[:, 0:8, :], PT[:, 8:16, :], mybir.AluOpType.add
            )
            t4 = trp.tile([128, 4, 512], BF16, tag="t4", name=f"t4_{qb}")
            nc.vector.tensor_tensor(
                t4[:], t8[:, 0:4, :], t8[:, 4:8, :], mybir.AluOpType.add
            )
            t2 = trp.tile([128, 2, 512], BF16, tag="t2", name=f"t2_{qb}")
            nc.vector.tensor_tensor(
                t2[:], t4[:, 0:2, :], t4[:, 2:4, :], mybir.AluOpType.add
            )
            PTsum = trp.tile([128, 512], BF16, tag="t1", name=f"t1_{qb}")
            nc.vector.tensor_tensor(
                PTsum[:], t2[:, 0, :], t2[:, 1, :], mybir.AluOpType.add
            )
            PTs.append(PT)
            PTsums.append(PTsum)

        def emit_sums(qb):
            """partition-reduce PTsum via 1-wide ones-matmuls + reciprocal"""
            ps_sum = psSum.tile([128, 4], F32, tag="psSum")
            recs = statp.tile([128, 4], F32, tag="recs", name=f"recs{qb}")
            for j in range(4):
                nc.tensor.matmul(
                    ps_sum[:, j : j + 1],
                    PTsums[qb][:, j * 128 : (j + 1) * 128],
                    ones[:],
                    start=True, stop=True, skip_group_check=True,
                )
            nc.vector.reciprocal(recs[:], ps_sum[:])
            recss.append(recs)

        for qb in range(NQB):
            emit_scores(qb)
        emit_sums(0)

        UTs = []

        def emit_u(qb):
            """C2: U^T[d, q] = sum_kt dec_k^T-chunk @ PT."""
            UT = utp.tile([128, DS, 512], BF16, tag="utp", name=f"UT{qb}")
            # dj-outer: each U^T bank finishes its 16-kt chain early and
            # drains to SBUF while later banks accumulate, so the next
            # phase's PSUM reuse never waits on a burst of 8 casts
            for dj in range(DS):
                up = psU.tile([128, 512], F32, tag="psM", name=f"u{qb}_{dj}")
                for kt in range(NKT):
                    nc.tensor.matmul(
                        up[:], DK[:, kt, dj * 128 : (dj + 1) * 128],
                        PTs[qb][:, kt, :],
                        start=(kt == 0), stop=(kt == NKT - 1),
                    )
                nc.vector.tensor_copy(UT[:, dj, :], up[:])
            UTs.append(UT)

        emit_u(0)
        emit_sums(1)
        emit_u(1)

        def emit_out(qb):
            """C3: out[q, d] = (U @ Wkv_hi) * recip, per 128-q chunk."""
            UT, recs = UTs[qb], recss[qb]
            for qc in range(4):
                ot = ostp.tile([128, Dp], BF16, tag="ost")
                row0 = qb * 512 + qc * 128
                for dc in range(NDC):
                    ps = psO.tile([128, 512], F32, tag="psM")
                    for di in range(DS):
                        nc.tensor.matmul(
                            ps[:], UT[:, di, qc * 128 : (qc + 1) * 128],
                            whi[:, 4 * dc : 4 * (dc + 1), di, :],
                            start=(di == 0), stop=(di == DS - 1),
                        )
                    nc.scalar.activation(
                        ot[:, dc * 512 : (dc + 1) * 512], ps[:],
                        ACOPY, bias=0.0, scale=recs[:, qc : qc + 1],
                    )
                # one full-width DMA per q-chunk: 2KB contiguous per
                # partition row (vs 512B runs when column-split), and half
                # the DMA/semaphore count for the end-of-program drain
                nc.sync.dma_start(out[row0 : row0 + 128, :], ot[:])

        for qb in range(NQB):
            emit_out(qb)

        for cm in list(reversed(pools)):
            close(cm)

    legalize_waits(nc)
    return nc


def _pack_dT_blocks(x, DS):
    """[N, Dp] -> [N//128, 128, DS*128] where block b holds
    res[b, p, s*128+o] = x[b*128+o, s*128+p]."""
    N, Dp = x.shape
    r = x.reshape(N // 128, 128, DS, 128).transpose(0, 3, 2, 1)
    return np.ascontiguousarray(r.reshape(N // 128, 128, DS * 128))


def prepare_in_maps(hidden_states, decoder_hidden_states, Wq, Wkv):
    bf = ml_dtypes.bfloat16
    hs32 = np.asarray(hidden_states, dtype=np.float32)
    dec32 = np.asarray(decoder_hidden_states, dtype=np.float32)
    Wq32 = np.asarray(Wq, dtype=np.float32)
    Wkv32 = np.asarray(Wkv, dtype=np.float32)
    QS = QL // 2
    DS = D // 128

    w_qk = (Wq32 @ Wkv32[:, :D].T).astype(bf)     # fold Wq and Wkv_lo
    w_hi = Wkv32[:, D:].astype(bf)

    hidden_states = hs32.astype(bf)
    dec = dec32.astype(bf)

    wqk_p = _pack_dT_blocks(w_qk.T, DS)
    whi_p = _pack_dT_blocks(w_hi.T, DS)

    in_maps = []
    for c in range(N_CORES):
        b, h = c // 2, c % 2
        hs = hidden_states[b, h * QS : (h + 1) * QS]   # [QS, D]
        d_ = dec[b]                                    # [KL, D]
        in_maps.append(
            {
                "hsT": _pack_dT_blocks(hs, DS),
                "decT": _pack_dT_blocks(d_, DS),             # d-major blocks
                "deck": np.ascontiguousarray(d_.reshape(KL // 128, 128, D)),
                "wqk": wqk_p,
                "whi": whi_p,
            }
        )
    return in_maps


def kernel(hidden_states, decoder_hidden_states, Wq, Wkv):
    QS = QL // 2
    scale = 1.0 / float(np.sqrt(D))

    nc = bass.Bass()
    build_attention(nc, QS, KL, D, scale)
    in_maps = prepare_in_maps(hidden_states, decoder_hidden_states, Wq, Wkv)

    res = run_bass_kernel_spmd(nc, in_maps, list(range(N_CORES)))

    out = np.empty((B, QL, D), dtype=np.float32)
    for c in range(N_CORES):
        b, h = c // 2, c % 2
        out[b, h * QS : (h + 1) * QS] = np.asarray(res.results[c]["out"]).astype(
            np.float32
        )
    return out
